# revision 2
# baseline (speedup 1.0000x reference)
"""BiMPM matching-layer kernel v2 for Trainium2 (8 NeuronCores, pure data parallel).

Differences vs v1 baseline:
- all matmuls bf16 (norms via bf16 squares; errors wash out over H=128)
- fp32 inputs dropped entirely (half the input DMA traffic)
- m2 rep rows (1/n2 per (l,q)) broadcast to 128 partitions via DRAM round-trip
  DMA instead of fp32 PE matmuls + ACT copies
- m2 max-over-q via TT-max tree at bf16 2x instead of 1x TensorReduce
- A_SET perspectives take a transposed path: num_T matmul (q-partition), ACT
  per-partition scale drain, PE transpose back, DVE 2x copy [or ACT copy]
- batched tails (m1/m3/m4 in one 60-col stage), merged recips/sqrts
- w2 block order host-side: [m1 | m3 | m4 | m2 | ones] per direction

Output (B, S1, 8*L), L=20. Sharding: batch 64 -> 8/core.
"""
import numpy as np

B, S, H, L = 64, 256, 128, 20
NCORES = 8
BPC = B // NCORES
EPS = 1e-8

# l's (m2 perspective index, 0..19) handled by the transposed ACT path.
# Must be a contiguous range starting after D_COUNT for slicing simplicity:
# D-set = [0, D_COUNT), A-set = [D_COUNT, 20). A-set processed in groups of 2.
D_COUNT = 8
# Of the A-set, how many use ACT for the psT->SBUF copy-back (rest use DVE).
A_ACT_COPYBACK = 0

_cache = {}


def _build_bass():
    from contextlib import ExitStack

    import concourse.bass as bass
    import concourse.tile as tile
    from concourse import mybir

    f32 = mybir.dt.float32
    bf16 = mybir.dt.bfloat16
    AF = mybir.ActivationFunctionType
    OP = mybir.AluOpType

    nD = D_COUNT
    nA = L - nD
    assert nA % 2 == 0, "A-set must be even (groups of 2)"

    nc = bass.Bass()

    # DRAM I/O (per core) -- all bf16 except small fp32 weights
    pTb = nc.dram_tensor("pTb", [BPC, 2, H, S], bf16, kind="ExternalInput")
    qTb = nc.dram_tensor("qTb", [BPC, 2, H, S], bf16, kind="ExternalInput")
    qnb = nc.dram_tensor("qnb", [BPC, 2, S, H], bf16, kind="ExternalInput")
    q0nb = nc.dram_tensor("q0nb", [2, S, H], bf16, kind="ExternalInput")
    w2T = nc.dram_tensor("w2T", [2, H, 81], f32, kind="ExternalInput")
    w2Tb = nc.dram_tensor("w2Tb", [2, H, 81], bf16, kind="ExternalInput")
    iota2b = nc.dram_tensor("iota2b", [H, 2], bf16, kind="ExternalInput")
    onesrb = nc.dram_tensor("onesrb", [1, H], bf16, kind="ExternalInput")
    onescb = nc.dram_tensor("onescb", [H, 1], bf16, kind="ExternalInput")
    identb = nc.dram_tensor("identb", [H, H], bf16, kind="ExternalInput")
    out = nc.dram_tensor("out", [BPC, S, 8 * L], f32, kind="ExternalOutput")
    # DRAM scratch for partition-broadcast of per-(l,q) reciprocal norms:
    # row layout per (b,d): [r2flat (20*256) | rrow (256)]
    scr = nc.dram_tensor("scr", [BPC, 2, 21 * 256], bf16, kind="Internal")
    scr2 = nc.dram_tensor("scr2", [BPC, 2, 256], f32, kind="Internal")

    with tile.TileContext(nc) as tc, ExitStack() as ctx:
        cpool = ctx.enter_context(tc.tile_pool(name="consts", bufs=1))
        inp = ctx.enter_context(tc.tile_pool(name="inp", bufs=3))
        mid = ctx.enter_context(tc.tile_pool(name="mid", bufs=3))
        big = ctx.enter_context(tc.tile_pool(name="big", bufs=2))
        outp = ctx.enter_context(tc.tile_pool(name="outp", bufs=3))
        # one rotating pool for ALL psum tiles; lifetimes are kept short and
        # allocation order is lifetime-aware so the 8-bank FIFO never blocks long
        ps = ctx.enter_context(tc.tile_pool(name="ps", bufs=8, space="PSUM"))

        def pst(name="ps"):
            return ps.tile([H, 512], mybir.dt.float32, name="ps")

        # ---- constants ----
        w2f = cpool.tile([H, 2, 81], f32)
        nc.sync.dma_start(out=w2f, in_=w2T[:].rearrange("d h c -> h d c"))
        w2b = cpool.tile([H, 2, 81], bf16)
        nc.sync.dma_start(out=w2b, in_=w2Tb[:].rearrange("d h c -> h d c"))
        q0cb = cpool.tile([H, 2, 2, H], bf16)
        nc.sync.dma_start(out=q0cb, in_=q0nb[:].rearrange("d (c q) h -> q d c h", c=2))
        iotab = cpool.tile([H, 2], bf16)
        nc.sync.dma_start(out=iotab, in_=iota2b[:])
        onesr = cpool.tile([1, H], bf16)
        nc.sync.dma_start(out=onesr, in_=onesrb[:])
        onesc = cpool.tile([H, 1], bf16)
        nc.sync.dma_start(out=onesc, in_=onescb[:])
        onescf = cpool.tile([H, 1], f32)
        nc.vector.tensor_copy(onescf, onesc)
        ident = cpool.tile([H, H], bf16)
        nc.sync.dma_start(out=ident, in_=identb[:])
        # DVE warms (absorb const DMA sems once on DVE)
        warm = cpool.tile([H, 8], f32)
        warmb = cpool.tile([H, 8], bf16)
        nc.vector.tensor_copy(warm[:, 0:1], w2f[:, 0, 0:1])
        nc.vector.tensor_copy(warmb[:, 0:1], w2b[:, 0, 0:1])
        nc.vector.tensor_copy(warmb[:, 1:2], q0cb[:, 0, 0, 0:1])
        nc.vector.tensor_copy(warmb[:, 2:3], iotab[:, 0:1])
        nc.vector.tensor_copy(warmb[0:1, 3:4], onesr[0:1, 0:1])
        nc.vector.tensor_copy(warmb[:, 4:5], ident[:, 0:1])
        # PE const absorbs (chained)
        ab_w2 = nc.tensor.ldweights(w2b[:, 0, 0:1])
        ab_q0 = nc.tensor.ldweights(q0cb[:, 0, 0, 0:1])
        ab_id = nc.tensor.ldweights(ident[:, 0:1])
        ab_on = nc.tensor.ldweights(onesc[:, 0:1])
        bass._add_dep_helper(ab_q0.ins, ab_w2.ins, sync=False, reason="const chain")
        bass._add_dep_helper(ab_id.ins, ab_q0.ins, sync=False, reason="const chain")
        bass._add_dep_helper(ab_on.ins, ab_id.ins, sync=False, reason="const chain")

        def dep(from_inst, to_inst, why="order", sync=False):
            bass._add_dep_helper(from_inst.ins, to_inst.ins, sync=sync, reason=why)

        for b in range(BPC):
            # outt layout: (p128, chunk2, group4, dir2, 20); per chunk flattens to
            # the reference order [m1fw m1bw m2fw m2bw m3fw m3bw m4fw m4bw]
            outt = outp.tile([H, 2, 4, 2, L], f32, name="outt")
            for d in range(2):
                w2fd = w2f[:, d, :]
                w2bd = w2b[:, d, :]
                first = b == 0 and d == 0
                # ---- loads ----
                v1Tb = inp.tile([H, S], bf16, name="v1Tb")
                nc.sync.dma_start(out=v1Tb, in_=pTb[b, d])
                v2Tb = inp.tile([H, S], bf16, name="v2Tb")
                nc.sync.dma_start(out=v2Tb, in_=qTb[b, d])
                v2natb = inp.tile([H, 2, H], bf16, name="v2natb")
                nc.sync.dma_start(out=v2natb, in_=qnb[b, d].rearrange("(c q) h -> q c h", c=2))
                # absorb load sems once on DVE + PE
                dve_scr = mid.tile([H, 2], bf16, name="dve_scr")
                abs_nat = nc.vector.tensor_copy(dve_scr[:, 0:1], v2natb[:, 0, 0:1])
                abs_v1 = nc.vector.tensor_copy(dve_scr[:, 1:2], v1Tb[:, 0:1])
                ab1 = nc.tensor.ldweights(v1Tb[:, 0:1])
                ab2 = nc.tensor.ldweights(v2Tb[:, 0:1])
                dep(ab2, ab1, "pe absorb chain")
                if first:
                    dep(ab1, ab_on, "consts before pair absorbs")

                # ---- squares (bf16) ----
                v1sqb = mid.tile([H, S], bf16, name="v1sqb")
                nc.scalar.square(v1sqb, v1Tb)
                v2sqb = mid.tile([H, S], bf16, name="v2sqb")
                nc.scalar.square(v2sqb, v2Tb)
                v2sqf = mid.tile([H, S], f32, name="v2sqf")
                nc.scalar.square(v2sqf, v2Tf)

                # ---- norms: psNQ = [n1c0|n1c1|n2c0|n2c1] (81 each) + n2T (2x20) ----
                psNQ = pst()
                norm_mms = [
                    nc.tensor.matmul(psNQ[:, 0:81], v1sqb[:, 0:H], w2bd, start=True, stop=True),
                    nc.tensor.matmul(psNQ[:, 81:162], v1sqb[:, H:S], w2bd, start=True, stop=True),
                    nc.tensor.matmul(psNQ[:, 162:243], v2sqb[:, 0:H], w2bd, start=True, stop=True),
                    nc.tensor.matmul(psNQ[:, 243:324], v2sqb[:, H:S], w2bd, start=True, stop=True),
                ]
                if nA:
                    norm_mms.append(nc.tensor.matmul(
                        psNQ[:, 324:344], v2sqb[:, 0:H], w2bd[:, 60:80], start=True, stop=True))
                    norm_mms.append(nc.tensor.matmul(
                        psNQ[:, 344:364], v2sqb[:, H:S], w2bd[:, 60:80], start=True, stop=True))
                if first:
                    for mm in norm_mms:
                        dep(mm, ab_on, "consts before first mms")
                ncols = 324 + (40 if nA else 0)
                rsA = mid.tile([H, 364], f32, name="rsA")
                nc.vector.reciprocal(rsA[:, 0:ncols], psNQ[:, 0:ncols])
                nc.scalar.sqrt(rsA[:, 0:ncols], rsA[:, 0:ncols])
                nv1e = mid.tile([H, 2], f32, name="nv1e")
                nc.scalar.activation(nv1e, psNQ[:, 80:162:81], AF.Sqrt, scale=EPS * EPS)

                # ---- psQ: [n2bT (20 m2 row norms) | rrow] in one 21-row matmul ----
                psQ = pst()
                nc.tensor.matmul(psQ[0:20, 0:256], w2bd[:, 60:80], v2sqb, start=True, stop=True)
                nc.tensor.matmul(psQ[0:1, 256:512], onescf, v2sqf, start=True, stop=True)
                r2bTb = mid.tile([20, 256], bf16, name="r2bTb")
                r2tmp = mid.tile([20, 256], f32, name="r2tmp")
                nc.vector.reciprocal(r2tmp, psQ[0:20, 0:256])
                nc.scalar.sqrt(r2bTb, r2tmp)
                wr1 = nc.sync.dma_start(out=scr[b, d, 0:5120].rearrange("(l q) -> l q", l=20), in_=r2bTb)
                rrowf = mid.tile([1, 256], f32, name="rrowf")
                rrtmp = mid.tile([1, 256], f32, name="rrtmp")
                nc.vector.reciprocal(rrtmp, psQ[0:1, 256:512])
                nc.scalar.sqrt(rrowf, rrtmp)
                wr2 = nc.sync.dma_start(out=scr2[b, d].unsqueeze(0), in_=rrowf)
                rrep = mid.tile([H, 256], f32, name="rrep")
                rd2 = nc.sync.dma_start(out=rrep, in_=scr2[b, d].unsqueeze(0).to_broadcast((H, 256)))
                dep(rd2, wr2, "scr rrow RAW", sync=True)
                if nD:
                    repd = mid.tile([H, nD, 256], bf16, name="repd")
                    rd1 = nc.sync.dma_start(
                        out=repd,
                        in_=scr[b, d, 0:nD * 256].rearrange("(l q) -> l q", l=nD).unsqueeze(0).to_broadcast((H, nD, 256)))
                    dep(rd1, wr1, "scr reps RAW", sync=True)

                # ---- m2 scaled operands ----
                v1w = big.tile([H, L, S], bf16, name="v1w")
                for l in range(L):
                    nc.vector.tensor_scalar_mul(v1w[:, l, :], v1Tb, w2fd[:, 60 + l:61 + l])
                T128 = big.tile([H, 2, L, H], bf16, name="T128")
                T2d = big.tile([H, 2, nD, 256], bf16, name="T2d") if nD else None

                def a_chain(g):
                    """transposed-num chain for A-group g -> (psT_qc0, psT_qc1)"""
                    l0 = nD + 2 * g
                    psTs_qc = []
                    for qc in range(2):
                        psNumT = pst()
                        mmt = nc.tensor.matmul(
                            psNumT[:].rearrange("q (lg cp) -> q lg cp", lg=2),
                            v2Tb[:, qc * H:qc * H + H], v1w[:, l0:l0 + 2, :],
                            start=True, stop=True)
                        dep(mmt, ab2, "v2 absorbed before numT")
                        stT = mid.tile([H, 2, S], bf16, name="stT")
                        for j in range(2):
                            nc.scalar.activation(
                                stT[:, j, :], psNumT[:, j * 256:j * 256 + 256], AF.Copy,
                                scale=rsA[:, 324 + 20 * qc + l0 + j:324 + 20 * qc + l0 + j + 1])
                        psT = ps.tile([H, 2, 2, H], bf16, name="ps")  # (lg, pc, q)
                        for j in range(2):
                            for pc in range(2):
                                trm = nc.tensor.transpose(
                                    psT[:, j, pc, :], stT[:, j, pc * H:pc * H + H], ident)
                                if first and g == 0:
                                    dep(trm, ab_on, "ident absorbed before m2 transposes")
                        psTs_qc.append(psT)
                    return psTs_qc

                def a_copy(g, psTs_qc):
                    """fold qc by max during copy-back into T128"""
                    l0 = nD + 2 * g
                    src0 = psTs_qc[0][:].rearrange("p lg pc q -> p pc lg q")
                    src1 = psTs_qc[1][:].rearrange("p lg pc q -> p pc lg q")
                    T2a = mid.tile([H, 2, 2, H], bf16, name="T2a")
                    if 2 * g < A_ACT_COPYBACK:
                        nc.scalar.copy(T2a, src0)
                    else:
                        nc.vector.tensor_copy(T2a, src0)
                    nc.vector.tensor_tensor(T128[:, :, l0:l0 + 2, :], src1, T2a, op=OP.max)

                def d_run(ls):
                    for l in ls:
                        psNum = pst()
                        m0 = nc.tensor.matmul(psNum[:, 0:256], v1w[:, l, 0:H], v2Tb, start=True, stop=True)
                        m1_ = nc.tensor.matmul(psNum[:, 256:512], v1w[:, l, H:S], v2Tb, start=True, stop=True)
                        dep(m0, ab2, "v2 absorbed before num")
                        dep(m1_, ab2, "v2 absorbed before num")
                        nc.vector.tensor_tensor(
                            T2d[:, :, l, :], psNum[:].rearrange("p (c q) -> p c q", c=2),
                            repd[:, l, :].unsqueeze(1).to_broadcast((H, 2, 256)), op=OP.mult)

                # two waves: issue a-chains, cover their latency with d-path scales
                ngrp = nA // 2
                wave1 = list(range(ngrp // 2))
                wave2 = list(range(ngrp // 2, ngrp))
                dls = list(range(nD))
                psTs = {}
                for g in wave1:
                    psTs[g] = a_chain(g)
                d_run(dls[:nD // 2])
                for g in wave1:
                    a_copy(g, psTs.pop(g))
                for g in wave2:
                    psTs[g] = a_chain(g)
                d_run(dls[nD // 2:])
                for g in wave2:
                    a_copy(g, psTs.pop(g))
                if nD:
                    nc.vector.tensor_tensor(
                        T128[:, :, 0:nD, :], T2d[:, :, :, 0:H], T2d[:, :, :, H:S], op=OP.max)

                # ---- G / GT / attentive machinery ----
                psG = pst()
                psGT = pst()
                for mm in [
                    nc.tensor.matmul(psG[:, 0:256], v1Tb[:, 0:H], v2Tb, start=True, stop=True),
                    nc.tensor.matmul(psG[:, 256:512], v1Tb[:, H:S], v2Tb, start=True, stop=True),
                    nc.tensor.matmul(psGT[:, 0:256], v2Tb[:, 0:H], v1Tb, start=True, stop=True),
                    nc.tensor.matmul(psGT[:, 256:512], v2Tb[:, H:S], v1Tb, start=True, stop=True),
                ]:
                    dep(mm, ab2, "loads absorbed before G/GT")
                GT_sb = mid.tile([H, 2, S], bf16, name="GT_sb")
                nc.scalar.copy(GT_sb, psGT[:].rearrange("p (c q) -> p c q", c=2))
                v2rb = mid.tile([H, 2, H], bf16, name="v2rb")
                ts0 = nc.vector.tensor_scalar_mul(v2rb[:, 0, :], v2natb[:, 0, :], rsA[:, 242:243])
                ts1 = nc.vector.tensor_scalar_mul(v2rb[:, 1, :], v2natb[:, 1, :], rsA[:, 323:324])
                dep(ts0, abs_nat, "nat absorbed")
                dep(ts1, abs_nat, "nat absorbed")
                # argmax over q of G*rrow (f32: exact argmax + exact att3 sign)
                Gscf = mid.tile([H, 2, 256], f32, name="Gscf")
                nc.vector.tensor_tensor(
                    Gscf, psG[:].rearrange("p (c q) -> p c q", c=2),
                    rrep.unsqueeze(1).to_broadcast((H, 2, 256)), op=OP.mult)
                top8 = mid.tile([H, 2, 8], f32, name="top8")
                idx8 = mid.tile([H, 2, 8], mybir.dt.uint32, name="idx8")
                for c in range(2):
                    nc.vector.max(top8[:, c, :], Gscf[:, c, :])
                    nc.vector.max_index(idx8[:, c, :], top8[:, c, :], Gscf[:, c, :])
                idxfb = mid.tile([H, 2], bf16, name="idxfb")
                nc.vector.tensor_copy(idxfb, idx8[:, :, 0])
                GrS = mid.tile([H, 2], f32, name="GrS")
                nc.vector.tensor_reduce(GrS, Gscf, axis=mybir.AxisListType.X, op=OP.add)

                # ---- psSmall: m1 numerators, n2a, Gr, idxT carve, m3/m4 tails ----
                psSmall = pst()
                # m1
                tcol = v2Tb[:, 255:256] if d == 0 else v2Tb[:, 0:1]
                sqtb = mid.tile([H, 1], bf16, name="sqtb")
                nc.scalar.square(sqtb, tcol)
                tcolf = mid.tile([H, 1], f32, name="tcolf")
                nc.vector.tensor_copy(tcolf, tcol)
                rhs1b = mid.tile([H, 20], bf16, name="rhs1b")
                nc.vector.tensor_scalar_mul(rhs1b, w2bd[:, 0:20], tcolf)
                n2a_mm = nc.tensor.matmul(psSmall[0:1, 202:222], sqtb, w2bd[:, 0:20], start=True, stop=True)
                n2a_sb = mid.tile([1, 20], bf16, name="n2a_sb")
                nc.scalar.copy(n2a_sb, psSmall[0:1, 202:222])
                nc.tensor.matmul(psSmall[:, 222:242], onesr, n2a_sb, start=True, stop=True)
                for c in range(2):
                    mm = nc.tensor.matmul(
                        psSmall[:, 100 * c:100 * c + 20], v1Tb[:, c * H:c * H + H], rhs1b, start=True, stop=True)
                    dep(mm, ab1, "v1 absorbed before num1")
                # idx transposes into a bf16 carve of psSmall
                psIdxT = ps.tile([1, 256], bf16, name="ps")
                for c in range(2):
                    tr = nc.tensor.transpose(psIdxT[0:1, c * H:c * H + H], idxfb[:, c:c + 1], ident)
                    if first:
                        dep(tr, ab_on, "ident absorbed before transpose")
                idxTb = mid.tile([1, 256], bf16, name="idxTb")
                nc.scalar.copy(idxTb, psIdxT)
                # mean-attentive sign from the f32 row sums
                sgn = mid.tile([H, 2], f32, name="sgn")
                nc.scalar.activation(sgn[:, 0:1], GrS[:, 0:1], AF.Sign, bias=nv1e[:, 0:1], scale=1.0)
                nc.scalar.activation(sgn[:, 1:2], GrS[:, 1:2], AF.Sign, bias=nv1e[:, 1:2], scale=1.0)

                # ---- psIdx: argmax mask ----
                psIdx = pst()
                nc.tensor.matmul(psIdx[:, 0:256], onesr, idxTb, start=True, stop=True)
                maskb = mid.tile([H, 2, 256], bf16, name="maskb")
                nc.vector.tensor_tensor(
                    maskb, psIdx[:, 0:256].unsqueeze(1).to_broadcast((H, 2, 256)),
                    iotab.unsqueeze(2).to_broadcast((H, 2, 256)), op=OP.is_equal)

                # ---- psWork: mean-attentive (GWT) + max-attentive (att4T) ----
                psWork = pst()
                nc.tensor.matmul(psWork[:, 0:256], v2rb[:, 0, :], GT_sb[:, 0, :], start=True, stop=False)
                nc.tensor.matmul(psWork[:, 0:256], v2rb[:, 1, :], GT_sb[:, 1, :], start=False, stop=True)
                a4_mm0 = nc.tensor.matmul(psWork[:, 256:512], q0cb[:, d, 0, :], maskb[:, 0, :], start=True, stop=False)
                a4_mm1 = nc.tensor.matmul(psWork[:, 256:512], q0cb[:, d, 1, :], maskb[:, 1, :], start=False, stop=True)
                if first:
                    dep(a4_mm0, ab_on, "q0 absorbed before att4")
                    dep(a4_mm1, ab_on, "q0 absorbed before att4")
                prodsb = mid.tile([H, 2, S], bf16, name="prodsb")
                p_tt = nc.vector.tensor_tensor(
                    prodsb, psWork[:].rearrange("p (a q) -> p a q", a=2),
                    v1Tb.unsqueeze(1).to_broadcast((H, 2, S)), op=OP.mult)
                dep(p_tt, abs_v1, "v1 absorbed on DVE")
                sq34b = mid.tile([H, 2, S], bf16, name="sq34b")
                nc.scalar.square(sq34b, psWork[:].rearrange("p (a q) -> p a q", a=2))
                for c in range(2):
                    base = 100 * c
                    sl = slice(c * H, c * H + H)
                    nc.tensor.matmul(psSmall[:, base + 20:base + 40], prodsb[:, 0, sl], w2bd[:, 20:40], start=True, stop=True)
                    nc.tensor.matmul(psSmall[:, base + 40:base + 60], prodsb[:, 1, sl], w2bd[:, 40:60], start=True, stop=True)
                    nc.tensor.matmul(psSmall[:, base + 60:base + 80], sq34b[:, 0, sl], w2bd[:, 20:40], start=True, stop=True)
                    nc.tensor.matmul(psSmall[:, base + 80:base + 100], sq34b[:, 1, sl], w2bd[:, 40:60], start=True, stop=True)

                # ---- batched m1/m3/m4 tails ----
                mult60 = mid.tile([H, 2, 60], f32, name="mult60")
                nc.vector.reciprocal(mult60[:, :, 0:20], psSmall[:, 222:242].unsqueeze(1).to_broadcast((H, 2, 20)))
                nc.vector.reciprocal(
                    mult60[:, :, 20:60],
                    psSmall[:, 0:200].rearrange("p (c x) -> p c x", c=2)[:, :, 60:100])
                nc.scalar.sqrt(mult60, mult60)
                nc.vector.tensor_scalar_mul(mult60[:, 0, 20:40], mult60[:, 0, 20:40], sgn[:, 0:1])
                nc.vector.tensor_scalar_mul(mult60[:, 1, 20:40], mult60[:, 1, 20:40], sgn[:, 1:2])
                t134 = mid.tile([H, 2, 60], f32, name="t134")
                nc.vector.tensor_tensor(
                    t134, psSmall[:, 0:200].rearrange("p (c x) -> p c x", c=2)[:, :, 0:60],
                    rsA[:, 0:162].rearrange("p (c x) -> p c x", c=2)[:, :, 0:60], op=OP.mult)
                nc.vector.tensor_tensor(outt[:, :, 0, d, :], t134[:, :, 0:20], mult60[:, :, 0:20], op=OP.mult)
                nc.vector.tensor_tensor(outt[:, :, 2, d, :], t134[:, :, 20:40], mult60[:, :, 20:40], op=OP.mult)
                nc.vector.tensor_tensor(outt[:, :, 3, d, :], t134[:, :, 40:60], mult60[:, :, 40:60], op=OP.mult)

                # ---- m2 tree: 128 -> 1 ----
                cur = T128
                w = H
                lvl = 0
                while w > 1:
                    nxt = big.tile([H, 2, L, w // 2], bf16, name=f"tr{lvl}")
                    nc.vector.tensor_tensor(
                        nxt, cur[:, :, :, 0:w // 2], cur[:, :, :, w // 2:w], op=OP.max)
                    cur = nxt
                    w //= 2
                    lvl += 1
                nc.vector.tensor_tensor(
                    outt[:, :, 1, d, :], cur[:, :, :, 0],
                    rsA[:, 0:162].rearrange("p (c x) -> p c x", c=2)[:, :, 60:80], op=OP.mult)

            # ---- store ----
            nc.sync.dma_start(out=out[b, 0:H, :], in_=outt[:, 0].rearrange("p g d l -> p (g d l)"))
            nc.sync.dma_start(out=out[b, H:S, :], in_=outt[:, 1].rearrange("p g d l -> p (g d l)"))

    return nc


def _prep_core_inputs(p, q, w_list, core):
    """Host-side layout prep for one core. Only layout transforms + weight-only math."""
    import ml_dtypes

    bf = ml_dtypes.bfloat16
    sl = slice(core * BPC, (core + 1) * BPC)
    p8 = np.ascontiguousarray(p[sl])
    q8 = np.ascontiguousarray(q[sl])
    pT = np.ascontiguousarray(p8.reshape(BPC, S, 2, H).transpose(0, 2, 3, 1))
    qT = np.ascontiguousarray(q8.reshape(BPC, S, 2, H).transpose(0, 2, 3, 1))
    qn = np.ascontiguousarray(q8.reshape(BPC, S, 2, H).transpose(0, 2, 1, 3))
    q0n = np.ascontiguousarray(q[0].reshape(S, 2, H).transpose(1, 0, 2))

    # block order per direction: [m1 | m3 | m4 | m2 | ones]
    w2T = np.empty((2, H, 81), np.float32)
    for d in range(2):
        ws = w_list[d::2]  # [w1, w3, w5, w7] (fw) / [w2, w4, w6, w8] (bw)
        order = [0, 2, 3, 1]  # m1, m3, m4, m2
        cat = np.concatenate([ws[i] * ws[i] for i in order] + [np.ones((1, H), np.float32)], 0)
        w2T[d] = cat.T
    iota2 = np.stack([np.arange(H, dtype=np.float32), np.arange(H, 2 * H, dtype=np.float32)], 1)

    return {
        "pTb": pT.astype(bf),
        "qTb": qT.astype(bf),
        "qnb": qn.astype(bf),
        "q0nb": q0n.astype(bf),
        "w2T": w2T,
        "w2Tb": w2T.astype(bf),
        "iota2b": iota2.astype(bf),
        "onesrb": np.ones((1, H), bf),
        "onescb": np.ones((H, 1), bf),
        "identb": np.eye(H, dtype=np.float32).astype(bf),
    }
